# revision 2
# baseline (speedup 1.0000x reference)
"""Trainium2 Bass kernel v4 for nn_CustomAttention (B=4, N=2048, DIM=1024, 16h x 64).

Sharding: 8 cores = 4 batches x 2 head-groups (8 heads each). Host sums the
two partial y per batch (bf16 partials) + bias.

v4 schedule: paired-half attention units. For each head pair p and q-half,
the two heads (rows 0:64 / 64:128 of the kT/qT pair tiles) are processed
TOGETHER over the 16 key tiles:

  per kt step: S_A,S_B (4 matmuls, row-group packed -> ~2x PE throughput on
  the d=64 contraction), exp_A, exp_B on ACT (the critical engine; each exp's
  input is ready one exp-slot early -> no per-step semaphore wait), plus
  O-arc matmuls and projection queue pops filling the PE slack.

  O accumulation: one [65, HALF] PSUM accumulator shared by both halves via
  8-kt arcs (A:kt0-7 at step 8, B:kt0-7 at step 9, A:kt8-15 and B:kt8-15 at
  unit end), each arc drained (copy/add) by DVE into an SBUF accumulator.
  e-tiles persist in SBUF until their arc consumes them, which also absorbs
  early DMA latency.

PSUM: sA(2) + sB(2) single-buffered + o(2) + proj pj(2) = 8 banks.

DMA: inputs interleaved across the sync HWDGE and gpsimd SWDGE queues
(xt first-512-cols split out so the prologue projections start ~10us in);
y written as bf16 (host upcasts), alternating the two queues.
"""

import sys

sys.path.insert(0, '/opt/trn_rl_repo')

import numpy as np
import ml_dtypes

import concourse.bass as bass
import concourse.tile as tile
from concourse import bacc, mybir
from concourse.bass_utils import run_bass_kernel_spmd

B, N_TOK, DIM = 4, 2048, 1024
HEADS_TOTAL, D_HEAD = 16, 64
G_HEADS = 8              # heads per core
PAIRS = G_HEADS // 2     # head pairs per core
INNER_G = G_HEADS * D_HEAD   # 512, inner slice per core
SCALE = D_HEAD ** -0.5
F32 = mybir.dt.float32
BF16 = mybir.dt.bfloat16
BF16_NP = ml_dtypes.bfloat16

_NC_CACHE = {}

ARC = 8  # kt steps per O-accumulation arc


def build_kernel(n_tok=N_TOK, repeat=1, pops_per_step=4, ebufs=20):
    nc = bacc.Bacc("TRN2")
    xt = nc.declare_dram_parameter("xt", [DIM, n_tok], BF16, isOutput=False)
    wq = nc.declare_dram_parameter("wq", [DIM, INNER_G], BF16, isOutput=False)
    wk = nc.declare_dram_parameter("wk", [DIM, INNER_G], BF16, isOutput=False)
    wv = nc.declare_dram_parameter("wv", [DIM, INNER_G], BF16, isOutput=False)
    wo = nc.declare_dram_parameter("wo", [INNER_G, DIM], BF16, isOutput=False)
    y = nc.declare_dram_parameter("y", [n_tok, DIM], BF16, isOutput=True)

    KD = DIM // 128          # 8 contraction tiles for projections
    NTT = n_tok // 128       # 128-wide token tiles
    HALF = n_tok // 2        # q-half processed per attention pair-unit
    KT = n_tok // 128        # key tiles in attention

    import contextlib

    with tile.TileContext(nc) as tc:
      with (tc.For_i(0, repeat, 1) if repeat > 1 else contextlib.nullcontext()):
        with tc.tile_pool(name="main", bufs=1) as mp, \
             tc.tile_pool(name="epool", bufs=ebufs) as ep, \
             tc.tile_pool(name="osb", bufs=3) as osb_pool, \
             tc.tile_pool(name="norm", bufs=1) as npool, \
             tc.tile_pool(name="ypool", bufs=2) as ypool, \
             tc.tile_pool(name="ps_sa", bufs=1, space="PSUM") as ps_sa, \
             tc.tile_pool(name="ps_sb", bufs=1, space="PSUM") as ps_sb, \
             tc.tile_pool(name="ps_o", bufs=1, space="PSUM") as ps_o, \
             tc.tile_pool(name="ps_pj", bufs=2, space="PSUM") as ps_pj:

            # xt split: first 512 token-cols in xtA (prologue starts early),
            # rest in xtB.
            xtA = [mp.tile([128, 512], BF16, name=f"xtA{i}") for i in range(KD)]
            xtB = [mp.tile([128, n_tok - 512], BF16, name=f"xtB{i}")
                   for i in range(KD)]
            wq_sb = [mp.tile([128, INNER_G], BF16, name=f"wq{i}") for i in range(KD)]
            wk_sb = [mp.tile([128, INNER_G], BF16, name=f"wk{i}") for i in range(KD)]
            wv_sb = [mp.tile([128, INNER_G], BF16, name=f"wv{i}") for i in range(KD)]
            wo_sb = [mp.tile([128, DIM], BF16, name=f"wo{j}") for j in range(PAIRS)]
            qT = [mp.tile([128, n_tok], BF16, name=f"qT{p}") for p in range(PAIRS)]
            kT = [mp.tile([128, n_tok], BF16, name=f"kT{p}") for p in range(PAIRS)]
            vb = [mp.tile([128, G_HEADS, D_HEAD + 1], BF16, name=f"vb{t}")
                  for t in range(NTT)]
            aT = [mp.tile([128, n_tok], BF16, name=f"aT{p}") for p in range(PAIRS)]
            ones8 = mp.tile([128, G_HEADS], BF16, name="ones8")

            def xt_sl(i, c0, c1):
                """xt_sb[i][:, c0:c1]; [c0,c1) must not cross col 512."""
                if c1 <= 512:
                    return xtA[i][:, c0:c1]
                assert c0 >= 512
                return xtB[i][:, c0 - 512:c1 - 512]

            # ---- input DMAs, balanced across sync HWDGE + gpsimd SWDGE.
            # Per-queue order = arrival priority: xtA, wk, wq, xtB, wv, wo.
            def dma_in(use_sync, out, in_):
                eng = nc.sync if use_sync else nc.gpsimd
                eng.dma_start(out=out, in_=in_)

            for i in range(KD):
                dma_in(i % 2 == 0, xtA[i], xt[i * 128:(i + 1) * 128, 0:512])
            for i in range(KD):
                dma_in(i % 2 == 1, wk_sb[i], wk[i * 128:(i + 1) * 128, :])
            for i in range(KD):
                dma_in(i % 2 == 0, wq_sb[i], wq[i * 128:(i + 1) * 128, :])
            for i in range(KD):
                dma_in(i % 2 == 1, xtB[i], xt[i * 128:(i + 1) * 128, 512:])
            for i in range(KD):
                dma_in(i % 2 == 0, wv_sb[i], wv[i * 128:(i + 1) * 128, :])
            for j in range(PAIRS):
                dma_in(j % 2 == 1, wo_sb[j], wo[j * 128:(j + 1) * 128, :])
            nc.vector.memset(ones8, 1.0)

            # ---- projection work items ----
            def qk_chunk(w_sb, dst, p, qc, st, dname):
                for i in range(KD):
                    def item(i=i, w_sb=w_sb, dst=dst, qc=qc, st=st, p=p,
                             dname=dname):
                        if i == 0:
                            st['ps'] = ps_pj.tile(
                                [128, 512], F32, tag="pj",
                                name=f"pj_{dname}_{qc}")
                        nc.tensor.matmul(
                            out=st['ps'],
                            lhsT=w_sb[i][:, p * 128:(p + 1) * 128],
                            rhs=xt_sl(i, qc * 512, (qc + 1) * 512),
                            start=(i == 0), stop=(i == KD - 1))
                        if i == KD - 1:
                            nc.vector.tensor_copy(
                                out=dst[:, qc * 512:(qc + 1) * 512],
                                in_=st['ps'])
                    yield item

            def vproj_tile(t, st):
                for i in range(KD):
                    def item(i=i, t=t, st=st):
                        if i == 0:
                            st['ps'] = ps_pj.tile([128, 512], F32, tag="pj",
                                                  name=f"pj_v_{t}")
                        nc.tensor.matmul(
                            out=st['ps'],
                            lhsT=xt_sl(i, t * 128, (t + 1) * 128),
                            rhs=wv_sb[i],
                            start=(i == 0), stop=(i == KD - 1))
                        if i == KD - 1:
                            nc.vector.tensor_copy(out=vb[t][:, :, D_HEAD],
                                                  in_=ones8)
                            nc.vector.tensor_copy(
                                out=vb[t][:, :, 0:D_HEAD],
                                in_=st['ps'].rearrange("p (h d) -> p h d",
                                                       h=G_HEADS))
                    yield item

            def outproj_tile(t, jorder, st):
                for dc in range(2):
                    for jj, j in enumerate(jorder):
                        def item(t=t, dc=dc, j=j, jj=jj, st=st):
                            if dc == 0 and jj == 0:
                                st['y'] = ypool.tile([128, DIM], BF16, tag="y",
                                                     name=f"y_{t}")
                            if jj == 0:
                                st['ps'] = ps_pj.tile(
                                    [128, 512], F32, tag="pj",
                                    name=f"pj_out_{t}_{dc}")
                            nc.tensor.matmul(
                                out=st['ps'],
                                lhsT=aT[j][:, t * 128:(t + 1) * 128],
                                rhs=wo_sb[j][:, dc * 512:(dc + 1) * 512],
                                start=(jj == 0), stop=(jj == PAIRS - 1))
                            if jj == PAIRS - 1:
                                nc.vector.tensor_copy(
                                    out=st['y'][:, dc * 512:(dc + 1) * 512],
                                    in_=st['ps'])
                                if dc == 1:
                                    eng = nc.sync if t % 2 == 0 else nc.gpsimd
                                    eng.dma_start(
                                        out=y[t * 128:(t + 1) * 128, :],
                                        in_=st['y'])
                        yield item

            # work queue with tags; items MUST be emitted (popped) before any
            # consumer of their output is emitted — Tile dependency tracking
            # follows emission order, so a consumer emitted first would read
            # garbage. pop_until(tag) force-drains through the last item of
            # that tag.
            queue = []
            tag_remaining = {}

            def push(tag, items):
                for it in items:
                    queue.append((tag, it))
                    tag_remaining[tag] = tag_remaining.get(tag, 0) + 1

            def pop(n):
                for _ in range(min(n, len(queue))):
                    tag, fn = queue.pop(0)
                    tag_remaining[tag] -= 1
                    fn()

            def pop_until(tag):
                while tag_remaining.get(tag, 0) > 0:
                    pop(1)

            # ---- prologue: kT0 chunk0 + qT0 chunk0+1 inline ----
            for it in qk_chunk(wk_sb, kT[0], 0, 0, {}, "kT0"):
                it()
            for qc in range(2):
                for it in qk_chunk(wq_sb, qT[0], 0, qc, {}, "qT0"):
                    it()

            # queue: rest of kT0, V-proj (needed by unit0 arcs at step 8),
            # then pairs 1-3 kT + qT(first half), then all qT second halves
            # (needed by qh=1 units in reverse pair order).
            for qc in range(1, 4):
                push(f"kT0c{qc}", qk_chunk(wk_sb, kT[0], 0, qc, {}, "kT0"))
            for t in range(NTT):
                push(f"vb{t}", vproj_tile(t, {}))
            for p in range(1, PAIRS):
                for qc in range(4):
                    push(f"kT{p}c{qc}",
                         qk_chunk(wk_sb, kT[p], p, qc, {}, f"kT{p}"))
                for qc in range(2):
                    push(f"qT{p}a",
                         qk_chunk(wq_sb, qT[p], p, qc, {}, f"qT{p}"))
            for p in (3, 2, 1, 0):
                for qc in range(2, 4):
                    push(f"qT{p}b",
                         qk_chunk(wq_sb, qT[p], p, qc, {}, f"qT{p}b"))

            # ---- paired attention units ----
            # Arc plan per unit (o psum = single shared accumulator; per-step
            # pacing avoids multi-us PE bursts that would stall the exp
            # stream, and the B2 arc carries into the next unit so unit
            # boundaries stay seamless):
            #   step 0   : prev B2 first half (carry chunk 0)
            #   step 1   : prev B2 second half + drain + its normalize;
            #              open A1, O(kt0)
            #   steps 2-7: A1 O(kt-1)
            #   step 8   : A1 O(kt7) + drain; open B1, O(kt0-3)
            #   step 9   : B1 O(kt4-7) + drain
            #   step 10  : open A2, O(kt8) O(kt9)
            #   steps 11-14: A2 O(kt-2)
            #   step 15  : A2 O(kt14) O(kt15) + drain + normalize(0)
            #   B2 (kt8-15) -> returned as carry chunks
            # Emission order inside a step: 4 S matmuls (64-row, alternating
            # row groups -> concurrent), exps, then all full-row matmuls (O
            # arcs + projection pops) contiguously -- row-shape transitions
            # on the PE stream cost ~100-200ns each (LDWEIGHTS of a
            # conflicting row group cannot overlap an in-flight matmul), so
            # the stream is organized as one 64-row run + one 128-row run
            # per step.
            def attn_pair(p, qh, carry_in):
                q0 = qh * HALF
                e_tiles = {0: [None] * KT, 1: [None] * KT}
                o_sb = {}
                arc = {}

                def s_exp(kt_i):
                    psA = ps_sa.tile([128, HALF], F32, tag="sa",
                                     name=f"sa_{p}_{qh}_{kt_i}")
                    psB = ps_sb.tile([128, HALF], F32, tag="sb",
                                     name=f"sb_{p}_{qh}_{kt_i}")
                    for c in range(2):
                        for hl, ps in ((0, psA), (1, psB)):
                            po = hl * 64
                            nc.tensor.matmul(
                                out=ps[:, c * 512:(c + 1) * 512],
                                lhsT=kT[p][po:po + 64,
                                           kt_i * 128:(kt_i + 1) * 128],
                                rhs=qT[p][po:po + 64,
                                          q0 + c * 512:q0 + (c + 1) * 512],
                                start=True, stop=True)
                    for hl, ps in ((0, psA), (1, psB)):
                        et = ep.tile([128, HALF], BF16, tag="e",
                                     name=f"e_{p}_{qh}_{hl}_{kt_i}")
                        nc.scalar.activation(
                            out=et, in_=ps,
                            func=mybir.ActivationFunctionType.Exp,
                            scale=SCALE)
                        e_tiles[hl][kt_i] = et

                def arc_open(hl, arc_i):
                    arc['ps'] = ps_o.tile([D_HEAD + 1, HALF], F32, tag="o",
                                          name=f"o_{p}_{qh}_{hl}_{arc_i}")
                    arc['hl'], arc['i'] = hl, arc_i

                def arc_mm(kt_i, first, last):
                    pop_until(f"vb{kt_i}")
                    hl = arc['hl']
                    for c in range(2):
                        nc.tensor.matmul(
                            out=arc['ps'][:, c * 512:(c + 1) * 512],
                            lhsT=vb[kt_i][:, 2 * p + hl, :],
                            rhs=e_tiles[hl][kt_i][:, c * 512:(c + 1) * 512],
                            start=first, stop=last)

                def arc_drain():
                    hl = arc['hl']
                    if arc['i'] == 0:
                        o_sb[hl] = osb_pool.tile([D_HEAD + 1, HALF], F32,
                                                 tag="osb",
                                                 name=f"osb_{p}_{qh}_{hl}")
                        nc.vector.tensor_copy(out=o_sb[hl], in_=arc['ps'])
                    else:
                        nc.vector.tensor_add(o_sb[hl], o_sb[hl], arc['ps'])

                def normalize(hl):
                    ot = o_sb[hl]
                    r1 = npool.tile([1, HALF], F32, tag="r1",
                                    name=f"r1_{p}_{qh}_{hl}")
                    nc.sync.dma_start(out=r1, in_=ot[64:65, :])
                    r2 = npool.tile([1, HALF], F32, tag="r2",
                                    name=f"r2_{p}_{qh}_{hl}")
                    nc.vector.reciprocal(out=r2, in_=r1)
                    rb = npool.tile([64, HALF], F32, tag="rb",
                                    name=f"rb_{p}_{qh}_{hl}")
                    nc.gpsimd.partition_broadcast(rb, r2)
                    if hl == 0:
                        nc.vector.tensor_mul(
                            aT[p][0:64, q0:q0 + HALF], ot[0:64, :], rb)
                    else:
                        tmpb = npool.tile([64, HALF], BF16, tag="tmpb",
                                          name=f"tmpb_{p}_{qh}")
                        nc.vector.tensor_mul(tmpb, ot[0:64, :], rb)
                        nc.sync.dma_start(
                            out=aT[p][64:128, q0:q0 + HALF], in_=tmpb)

                for kt_i in range(KT):
                    if p > 0 or kt_i > 0:
                        pop_until(f"kT{p}c{kt_i // 4}")
                    if qh == 1:
                        pop_until(f"qT{p}b")
                    elif p > 0:
                        pop_until(f"qT{p}a")
                    s_exp(kt_i)
                    if kt_i < len(carry_in):
                        carry_in[kt_i]()
                    if kt_i == 1:
                        arc_open(0, 0)
                        arc_mm(0, True, False)
                    elif 2 <= kt_i <= 7:
                        arc_mm(kt_i - 1, False, False)
                    elif kt_i == 8:
                        arc_mm(7, False, True)
                        arc_drain()
                        arc_open(1, 0)
                        arc_mm(0, True, False)
                        arc_mm(1, False, False)
                    elif kt_i == 9:
                        for k2 in (2, 3, 4):
                            arc_mm(k2, False, False)
                    elif kt_i == 10:
                        for k2 in (5, 6):
                            arc_mm(k2, False, False)
                        arc_mm(7, False, True)
                        arc_drain()
                    elif kt_i == 11:
                        arc_open(0, 1)
                        arc_mm(8, True, False)
                        arc_mm(9, False, False)
                        arc_mm(10, False, False)
                    elif 12 <= kt_i <= 14:
                        arc_mm(kt_i - 1, False, False)
                    elif kt_i == 15:
                        arc_mm(14, False, False)
                        arc_mm(15, False, True)
                        arc_drain()
                        normalize(0)
                    pop(pops_budget[(qh, p)])

                def b2_first():
                    arc_open(1, 1)
                    for k2 in range(ARC, ARC + 4):
                        arc_mm(k2, k2 == ARC, False)

                def b2_second():
                    for k2 in range(ARC + 4, KT):
                        arc_mm(k2, False, k2 == KT - 1)
                    arc_drain()
                    normalize(1)

                return [b2_first, b2_second]

            pops_budget = {(0, 0): 10, (0, 1): 3, (0, 2): 3, (0, 3): 3,
                           (1, 0): 2, (1, 1): 2, (1, 2): 2, (1, 3): 2}
            if isinstance(pops_per_step, dict):
                pops_budget.update(pops_per_step)

            carry = []
            for qh in range(2):
                for u in range(PAIRS):
                    p = u if qh == 0 else PAIRS - 1 - u
                    carry = attn_pair(p, qh, carry)
                if qh == 0:
                    for t in range(NTT // 2):
                        push(f"op{t}", outproj_tile(t, (0, 1, 2, 3), {}))

            # ---- epilogue: last B2 arc + normalize, then qh=1 out-proj ----
            for ch in carry:
                ch()
            for t in range(NTT // 2, NTT):
                push(f"op{t}", outproj_tile(t, (3, 2, 1, 0), {}))
            pop(len(queue))

    nc.compile()
    return nc


def kernel(x, w_qkv, w_out, b_out):
    x = np.asarray(x, dtype=np.float32)
    w_qkv = np.asarray(w_qkv, dtype=np.float32)
    w_out = np.asarray(w_out, dtype=np.float32)
    b_out = np.asarray(b_out, dtype=np.float32)

    if N_TOK not in _NC_CACHE:
        _NC_CACHE[N_TOK] = build_kernel(N_TOK)
    nc = _NC_CACHE[N_TOK]

    core_ids = list(range(8))
    in_maps = _make_in_maps(x, w_qkv, w_out)
    res = run_bass_kernel_spmd(nc, in_maps, core_ids)
    out = np.empty((B, N_TOK, DIM), dtype=np.float32)
    for b in range(B):
        out[b] = (res.results[2 * b]["y"].astype(np.float32)
                  + res.results[2 * b + 1]["y"].astype(np.float32) + b_out)
    return out


def _make_in_maps(x, w_qkv, w_out):
    in_maps = []
    for c in range(8):
        b, g = c // 2, c % 2
        sl = slice(g * INNER_G, (g + 1) * INNER_G)
        in_maps.append({
            "xt": np.ascontiguousarray(x[b].T).astype(BF16_NP),
            "wq": np.ascontiguousarray(
                w_qkv[:, 0 * DIM + sl.start:0 * DIM + sl.stop]).astype(BF16_NP),
            "wk": np.ascontiguousarray(
                w_qkv[:, 1 * DIM + sl.start:1 * DIM + sl.stop]).astype(BF16_NP),
            "wv": np.ascontiguousarray(
                w_qkv[:, 2 * DIM + sl.start:2 * DIM + sl.stop]).astype(BF16_NP),
            "wo": np.ascontiguousarray(w_out[sl]).astype(BF16_NP),
        })
    return in_maps


# revision 3
# speedup vs baseline: 1.4421x; 1.4421x over previous
"""Trainium2 Bass kernel v4 for nn_CustomAttention (B=4, N=2048, DIM=1024, 16h x 64).

Sharding: 8 cores = 4 batches x 2 head-groups (8 heads each). Host sums the
two partial y per batch (bf16 partials) + bias.

v4.1 schedule: paired-half attention units. For each head pair p and q-half,
the two heads (rows 0:64 / 64:128 of the kT/qT pair tiles) are processed
TOGETHER over the 16 key tiles:

  per kt step: S_A,S_B (4 matmuls alternating PE row groups -> LDWEIGHTS and
  the matmuls themselves overlap across groups, ~111 ns/MM vs 419 same-group),
  exp_A, exp_B on ACT (the bottleneck engine, ~1234 ns per [128,1024] exp from
  PSUM; each exp's input is ready one exp-slot early so ACT never waits in
  steady state), then ALL full-row matmuls (O arcs + projection queue pops)
  contiguously -- only 2 row-shape transitions per step, since a 64-row and a
  128-row matmul serialize each other's weight loads.

  O accumulation: one [65, HALF] PSUM accumulator shared by both halves via
  8-kt arcs paced ~2 MMs/step (A1 steps 1-8, B1 8-10, A2 11-15); the last
  (B2) arc carries into the NEXT unit's steps 0-1 so unit boundaries never
  burst. Each arc is drained (copy/add) by DVE into an SBUF accumulator;
  e-tiles persist in SBUF until their arc consumes them.

Projections flow through a tagged work queue; items must be POPPED (emitted)
before any consumer instruction is emitted -- Tile dependencies follow
emission order -- enforced by pop_until(tag) deadlines + per-unit budgets.

PSUM: sA(2) + sB(2) single-buffered + o(2) + proj pj(2) = 8 banks.

DMA: inputs interleaved across the sync HWDGE and gpsimd SWDGE queues
(~110 GB/s each; the scalar HWDGE is slow -- unused). xt's first 512
token-cols are a separate tile so prologue projections start ~10us in.
y written as bf16 (host upcasts + sums partials), alternating both queues.
"""

import sys

sys.path.insert(0, '/opt/trn_rl_repo')

import numpy as np
import ml_dtypes

import concourse.bass as bass
import concourse.tile as tile
from concourse import bacc, mybir
from concourse.bass_utils import run_bass_kernel_spmd

B, N_TOK, DIM = 4, 2048, 1024
HEADS_TOTAL, D_HEAD = 16, 64
G_HEADS = 8              # heads per core
PAIRS = G_HEADS // 2     # head pairs per core
INNER_G = G_HEADS * D_HEAD   # 512, inner slice per core
SCALE = D_HEAD ** -0.5
F32 = mybir.dt.float32
BF16 = mybir.dt.bfloat16
BF16_NP = ml_dtypes.bfloat16

_NC_CACHE = {}

ARC = 8  # kt steps per O-accumulation arc


def build_kernel(n_tok=N_TOK, repeat=1, pops_per_step=4, ebufs=20):
    nc = bacc.Bacc("TRN2")
    xt = nc.declare_dram_parameter("xt", [DIM, n_tok], BF16, isOutput=False)
    wq = nc.declare_dram_parameter("wq", [DIM, INNER_G], BF16, isOutput=False)
    wk = nc.declare_dram_parameter("wk", [DIM, INNER_G], BF16, isOutput=False)
    wv = nc.declare_dram_parameter("wv", [DIM, INNER_G], BF16, isOutput=False)
    wo = nc.declare_dram_parameter("wo", [INNER_G, DIM], BF16, isOutput=False)
    y = nc.declare_dram_parameter("y", [n_tok, DIM], BF16, isOutput=True)

    KD = DIM // 128          # 8 contraction tiles for projections
    NTT = n_tok // 128       # 128-wide token tiles
    HALF = n_tok // 2        # q-half processed per attention pair-unit
    KT = n_tok // 128        # key tiles in attention

    import contextlib

    with tile.TileContext(nc) as tc:
      with (tc.For_i(0, repeat, 1) if repeat > 1 else contextlib.nullcontext()):
        with tc.tile_pool(name="main", bufs=1) as mp, \
             tc.tile_pool(name="epool", bufs=ebufs) as ep, \
             tc.tile_pool(name="osb", bufs=3) as osb_pool, \
             tc.tile_pool(name="norm", bufs=1) as npool, \
             tc.tile_pool(name="ypool", bufs=2) as ypool, \
             tc.tile_pool(name="ps_sa", bufs=1, space="PSUM") as ps_sa, \
             tc.tile_pool(name="ps_sb", bufs=1, space="PSUM") as ps_sb, \
             tc.tile_pool(name="ps_o", bufs=1, space="PSUM") as ps_o, \
             tc.tile_pool(name="ps_pj", bufs=2, space="PSUM") as ps_pj:

            # xt split: first 512 token-cols in xtA (prologue starts early),
            # rest in xtB.
            xtA = [mp.tile([128, 512], BF16, name=f"xtA{i}") for i in range(KD)]
            xtB = [mp.tile([128, n_tok - 512], BF16, name=f"xtB{i}")
                   for i in range(KD)]
            wq_sb = [mp.tile([128, INNER_G], BF16, name=f"wq{i}") for i in range(KD)]
            wk_sb = [mp.tile([128, INNER_G], BF16, name=f"wk{i}") for i in range(KD)]
            wv_sb = [mp.tile([128, INNER_G], BF16, name=f"wv{i}") for i in range(KD)]
            wo_sb = [mp.tile([128, DIM], BF16, name=f"wo{j}") for j in range(PAIRS)]
            qT = [mp.tile([128, n_tok], BF16, name=f"qT{p}") for p in range(PAIRS)]
            kT = [mp.tile([128, n_tok], BF16, name=f"kT{p}") for p in range(PAIRS)]
            vb = [mp.tile([128, G_HEADS, D_HEAD + 1], BF16, name=f"vb{t}")
                  for t in range(NTT)]
            aT = [mp.tile([128, n_tok], BF16, name=f"aT{p}") for p in range(PAIRS)]
            ones8 = mp.tile([128, G_HEADS], BF16, name="ones8")

            def xt_sl(i, c0, c1):
                """xt_sb[i][:, c0:c1]; [c0,c1) must not cross col 512."""
                if c1 <= 512:
                    return xtA[i][:, c0:c1]
                assert c0 >= 512
                return xtB[i][:, c0 - 512:c1 - 512]

            # ---- input DMAs, balanced across sync HWDGE + gpsimd SWDGE.
            # Per-queue order = arrival priority: xtA, wk, wq, xtB, wv, wo.
            def dma_in(use_sync, out, in_):
                eng = nc.sync if use_sync else nc.gpsimd
                eng.dma_start(out=out, in_=in_)

            for i in range(KD):
                dma_in(i % 2 == 0, xtA[i], xt[i * 128:(i + 1) * 128, 0:512])
            for i in range(KD):
                dma_in(i % 2 == 1, wk_sb[i], wk[i * 128:(i + 1) * 128, :])
            for i in range(KD):
                dma_in(i % 2 == 0, wq_sb[i], wq[i * 128:(i + 1) * 128, :])
            for i in range(KD):
                dma_in(i % 2 == 1, xtB[i], xt[i * 128:(i + 1) * 128, 512:])
            for i in range(KD):
                dma_in(i % 2 == 0, wv_sb[i], wv[i * 128:(i + 1) * 128, :])
            for j in range(PAIRS):
                dma_in(j % 2 == 1, wo_sb[j], wo[j * 128:(j + 1) * 128, :])
            nc.vector.memset(ones8, 1.0)

            # ---- projection work items ----
            def qk_chunk(w_sb, dst, p, qc, st, dname):
                for i in range(KD):
                    def item(i=i, w_sb=w_sb, dst=dst, qc=qc, st=st, p=p,
                             dname=dname):
                        if i == 0:
                            st['ps'] = ps_pj.tile(
                                [128, 512], F32, tag="pj",
                                name=f"pj_{dname}_{qc}")
                        nc.tensor.matmul(
                            out=st['ps'],
                            lhsT=w_sb[i][:, p * 128:(p + 1) * 128],
                            rhs=xt_sl(i, qc * 512, (qc + 1) * 512),
                            start=(i == 0), stop=(i == KD - 1))
                        if i == KD - 1:
                            nc.vector.tensor_copy(
                                out=dst[:, qc * 512:(qc + 1) * 512],
                                in_=st['ps'])
                    yield item

            def vproj_tile(t, st):
                for i in range(KD):
                    def item(i=i, t=t, st=st):
                        if i == 0:
                            st['ps'] = ps_pj.tile([128, 512], F32, tag="pj",
                                                  name=f"pj_v_{t}")
                        nc.tensor.matmul(
                            out=st['ps'],
                            lhsT=xt_sl(i, t * 128, (t + 1) * 128),
                            rhs=wv_sb[i],
                            start=(i == 0), stop=(i == KD - 1))
                        if i == KD - 1:
                            nc.vector.tensor_copy(out=vb[t][:, :, D_HEAD],
                                                  in_=ones8)
                            nc.vector.tensor_copy(
                                out=vb[t][:, :, 0:D_HEAD],
                                in_=st['ps'].rearrange("p (h d) -> p h d",
                                                       h=G_HEADS))
                    yield item

            def outproj_tile(t, jorder, st):
                for dc in range(2):
                    for jj, j in enumerate(jorder):
                        def item(t=t, dc=dc, j=j, jj=jj, st=st):
                            if dc == 0 and jj == 0:
                                st['y'] = ypool.tile([128, DIM], BF16, tag="y",
                                                     name=f"y_{t}")
                            if jj == 0:
                                st['ps'] = ps_pj.tile(
                                    [128, 512], F32, tag="pj",
                                    name=f"pj_out_{t}_{dc}")
                            nc.tensor.matmul(
                                out=st['ps'],
                                lhsT=aT[j][:, t * 128:(t + 1) * 128],
                                rhs=wo_sb[j][:, dc * 512:(dc + 1) * 512],
                                start=(jj == 0), stop=(jj == PAIRS - 1))
                            if jj == PAIRS - 1:
                                nc.vector.tensor_copy(
                                    out=st['y'][:, dc * 512:(dc + 1) * 512],
                                    in_=st['ps'])
                                if dc == 1:
                                    eng = nc.sync if t % 2 == 0 else nc.gpsimd
                                    eng.dma_start(
                                        out=y[t * 128:(t + 1) * 128, :],
                                        in_=st['y'])
                        yield item

            # work queue with tags; items MUST be emitted (popped) before any
            # consumer of their output is emitted — Tile dependency tracking
            # follows emission order, so a consumer emitted first would read
            # garbage. pop_until(tag) force-drains through the last item of
            # that tag.
            queue = []
            tag_remaining = {}

            def push(tag, items):
                for it in items:
                    queue.append((tag, it))
                    tag_remaining[tag] = tag_remaining.get(tag, 0) + 1

            def pop(n):
                for _ in range(min(n, len(queue))):
                    tag, fn = queue.pop(0)
                    tag_remaining[tag] -= 1
                    fn()

            def pop_until(tag):
                while tag_remaining.get(tag, 0) > 0:
                    pop(1)

            # ---- prologue: kT0 chunk0 + qT0 chunk0+1 inline ----
            for it in qk_chunk(wk_sb, kT[0], 0, 0, {}, "kT0"):
                it()
            for qc in range(2):
                for it in qk_chunk(wq_sb, qT[0], 0, qc, {}, "qT0"):
                    it()

            # queue: rest of kT0, V-proj (needed by unit0 arcs at step 8),
            # then pairs 1-3 kT + qT(first half), then all qT second halves
            # (needed by qh=1 units in reverse pair order).
            for qc in range(1, 4):
                push(f"kT0c{qc}", qk_chunk(wk_sb, kT[0], 0, qc, {}, "kT0"))
            for t in range(NTT):
                push(f"vb{t}", vproj_tile(t, {}))
            for p in range(1, PAIRS):
                for qc in range(4):
                    push(f"kT{p}c{qc}",
                         qk_chunk(wk_sb, kT[p], p, qc, {}, f"kT{p}"))
                for qc in range(2):
                    push(f"qT{p}a",
                         qk_chunk(wq_sb, qT[p], p, qc, {}, f"qT{p}"))
            for p in (3, 2, 1, 0):
                for qc in range(2, 4):
                    push(f"qT{p}b",
                         qk_chunk(wq_sb, qT[p], p, qc, {}, f"qT{p}b"))

            # ---- paired attention units ----
            # Arc plan per unit (o psum = single shared accumulator; per-step
            # pacing avoids multi-us PE bursts that would stall the exp
            # stream, and the B2 arc carries into the next unit so unit
            # boundaries stay seamless):
            #   step 0   : prev B2 first half (carry chunk 0)
            #   step 1   : prev B2 second half + drain + its normalize;
            #              open A1, O(kt0)
            #   steps 2-7: A1 O(kt-1)
            #   step 8   : A1 O(kt7) + drain; open B1, O(kt0-3)
            #   step 9   : B1 O(kt4-7) + drain
            #   step 10  : open A2, O(kt8) O(kt9)
            #   steps 11-14: A2 O(kt-2)
            #   step 15  : A2 O(kt14) O(kt15) + drain + normalize(0)
            #   B2 (kt8-15) -> returned as carry chunks
            # Emission order inside a step: 4 S matmuls (64-row, alternating
            # row groups -> concurrent), exps, then all full-row matmuls (O
            # arcs + projection pops) contiguously -- row-shape transitions
            # on the PE stream cost ~100-200ns each (LDWEIGHTS of a
            # conflicting row group cannot overlap an in-flight matmul), so
            # the stream is organized as one 64-row run + one 128-row run
            # per step.
            def attn_pair(p, qh, carry_in):
                q0 = qh * HALF
                e_tiles = {0: [None] * KT, 1: [None] * KT}
                o_sb = {}
                arc = {}

                def s_exp(kt_i):
                    psA = ps_sa.tile([128, HALF], F32, tag="sa",
                                     name=f"sa_{p}_{qh}_{kt_i}")
                    psB = ps_sb.tile([128, HALF], F32, tag="sb",
                                     name=f"sb_{p}_{qh}_{kt_i}")
                    for c in range(2):
                        for hl, ps in ((0, psA), (1, psB)):
                            po = hl * 64
                            nc.tensor.matmul(
                                out=ps[:, c * 512:(c + 1) * 512],
                                lhsT=kT[p][po:po + 64,
                                           kt_i * 128:(kt_i + 1) * 128],
                                rhs=qT[p][po:po + 64,
                                          q0 + c * 512:q0 + (c + 1) * 512],
                                start=True, stop=True)
                    for hl, ps in ((0, psA), (1, psB)):
                        et = ep.tile([128, HALF], BF16, tag="e",
                                     name=f"e_{p}_{qh}_{hl}_{kt_i}")
                        nc.scalar.activation(
                            out=et, in_=ps,
                            func=mybir.ActivationFunctionType.Exp,
                            scale=SCALE)
                        e_tiles[hl][kt_i] = et

                def arc_open(hl, arc_i):
                    arc['ps'] = ps_o.tile([D_HEAD + 1, HALF], F32, tag="o",
                                          name=f"o_{p}_{qh}_{hl}_{arc_i}")
                    arc['hl'], arc['i'] = hl, arc_i

                def arc_mm(kt_i, first, last):
                    pop_until(f"vb{kt_i}")
                    hl = arc['hl']
                    for c in range(2):
                        nc.tensor.matmul(
                            out=arc['ps'][:, c * 512:(c + 1) * 512],
                            lhsT=vb[kt_i][:, 2 * p + hl, :],
                            rhs=e_tiles[hl][kt_i][:, c * 512:(c + 1) * 512],
                            start=first, stop=last)

                def arc_drain():
                    hl = arc['hl']
                    if arc['i'] == 0:
                        o_sb[hl] = osb_pool.tile([D_HEAD + 1, HALF], F32,
                                                 tag="osb",
                                                 name=f"osb_{p}_{qh}_{hl}")
                        nc.vector.tensor_copy(out=o_sb[hl], in_=arc['ps'])
                    else:
                        nc.vector.tensor_add(o_sb[hl], o_sb[hl], arc['ps'])

                def normalize(hl):
                    ot = o_sb[hl]
                    r1 = npool.tile([1, HALF], F32, tag="r1",
                                    name=f"r1_{p}_{qh}_{hl}")
                    nc.sync.dma_start(out=r1, in_=ot[64:65, :])
                    r2 = npool.tile([1, HALF], F32, tag="r2",
                                    name=f"r2_{p}_{qh}_{hl}")
                    nc.vector.reciprocal(out=r2, in_=r1)
                    rb = npool.tile([64, HALF], F32, tag="rb",
                                    name=f"rb_{p}_{qh}_{hl}")
                    nc.gpsimd.partition_broadcast(rb, r2)
                    if hl == 0:
                        nc.vector.tensor_mul(
                            aT[p][0:64, q0:q0 + HALF], ot[0:64, :], rb)
                    else:
                        tmpb = npool.tile([64, HALF], BF16, tag="tmpb",
                                          name=f"tmpb_{p}_{qh}")
                        nc.vector.tensor_mul(tmpb, ot[0:64, :], rb)
                        nc.sync.dma_start(
                            out=aT[p][64:128, q0:q0 + HALF], in_=tmpb)

                for kt_i in range(KT):
                    if p > 0 or kt_i > 0:
                        pop_until(f"kT{p}c{kt_i // 4}")
                    if qh == 1:
                        pop_until(f"qT{p}b")
                    elif p > 0:
                        pop_until(f"qT{p}a")
                    s_exp(kt_i)
                    if kt_i < len(carry_in):
                        carry_in[kt_i]()
                    if kt_i == 1:
                        arc_open(0, 0)
                        arc_mm(0, True, False)
                    elif 2 <= kt_i <= 7:
                        arc_mm(kt_i - 1, False, False)
                    elif kt_i == 8:
                        arc_mm(7, False, True)
                        arc_drain()
                        arc_open(1, 0)
                        arc_mm(0, True, False)
                        arc_mm(1, False, False)
                    elif kt_i == 9:
                        for k2 in (2, 3, 4):
                            arc_mm(k2, False, False)
                    elif kt_i == 10:
                        for k2 in (5, 6):
                            arc_mm(k2, False, False)
                        arc_mm(7, False, True)
                        arc_drain()
                    elif kt_i == 11:
                        arc_open(0, 1)
                        arc_mm(8, True, False)
                        arc_mm(9, False, False)
                        arc_mm(10, False, False)
                    elif 12 <= kt_i <= 14:
                        arc_mm(kt_i - 1, False, False)
                    elif kt_i == 15:
                        arc_mm(14, False, False)
                        arc_mm(15, False, True)
                        arc_drain()
                        normalize(0)
                    pop(pops_budget[(qh, p)])

                def b2_first():
                    arc_open(1, 1)
                    for k2 in range(ARC, ARC + 4):
                        arc_mm(k2, k2 == ARC, False)

                def b2_second():
                    for k2 in range(ARC + 4, KT):
                        arc_mm(k2, False, k2 == KT - 1)
                    arc_drain()
                    normalize(1)

                return [b2_first, b2_second]

            pops_budget = {(0, 0): 10, (0, 1): 3, (0, 2): 3, (0, 3): 3,
                           (1, 0): 2, (1, 1): 2, (1, 2): 2, (1, 3): 2}
            if isinstance(pops_per_step, dict):
                pops_budget.update(pops_per_step)

            carry = []
            for qh in range(2):
                for u in range(PAIRS):
                    p = u if qh == 0 else PAIRS - 1 - u
                    carry = attn_pair(p, qh, carry)
                if qh == 0:
                    for t in range(NTT // 2):
                        push(f"op{t}", outproj_tile(t, (0, 1, 2, 3), {}))

            # ---- epilogue: last B2 arc + normalize, then qh=1 out-proj ----
            for ch in carry:
                ch()
            for t in range(NTT // 2, NTT):
                push(f"op{t}", outproj_tile(t, (3, 2, 1, 0), {}))
            pop(len(queue))

    nc.compile()
    return nc


def kernel(x, w_qkv, w_out, b_out):
    x = np.asarray(x, dtype=np.float32)
    w_qkv = np.asarray(w_qkv, dtype=np.float32)
    w_out = np.asarray(w_out, dtype=np.float32)
    b_out = np.asarray(b_out, dtype=np.float32)

    if N_TOK not in _NC_CACHE:
        _NC_CACHE[N_TOK] = build_kernel(N_TOK)
    nc = _NC_CACHE[N_TOK]

    core_ids = list(range(8))
    in_maps = _make_in_maps(x, w_qkv, w_out)
    res = run_bass_kernel_spmd(nc, in_maps, core_ids)
    out = np.empty((B, N_TOK, DIM), dtype=np.float32)
    for b in range(B):
        out[b] = (res.results[2 * b]["y"].astype(np.float32)
                  + res.results[2 * b + 1]["y"].astype(np.float32) + b_out)
    return out


def _make_in_maps(x, w_qkv, w_out):
    in_maps = []
    for c in range(8):
        b, g = c // 2, c % 2
        sl = slice(g * INNER_G, (g + 1) * INNER_G)
        in_maps.append({
            "xt": np.ascontiguousarray(x[b].T).astype(BF16_NP),
            "wq": np.ascontiguousarray(
                w_qkv[:, 0 * DIM + sl.start:0 * DIM + sl.stop]).astype(BF16_NP),
            "wk": np.ascontiguousarray(
                w_qkv[:, 1 * DIM + sl.start:1 * DIM + sl.stop]).astype(BF16_NP),
            "wv": np.ascontiguousarray(
                w_qkv[:, 2 * DIM + sl.start:2 * DIM + sl.stop]).astype(BF16_NP),
            "wo": np.ascontiguousarray(w_out[sl]).astype(BF16_NP),
        })
    return in_maps
